# revision 70
# baseline (speedup 1.0000x reference)
"""DecoderLSTM (Bahdanau attention + LSTM + vocab fc) on 8 Trainium2 cores.

Sharding: data-parallel over batch (64 -> 8 rows/core); the sequential scan
stays local per core; zero collectives. Host shards/casts/transposes inputs
and reassembles (f16 logits -> f32 + fc_b on host).

Feature-major recurrence: state h2T/c2T kept transposed [a-tiles, 8], so all
LSTM-side matmuls have n=8 (PE cost ~ output free size), and the pointwise
ops run on [128, 4, 8] tiles. Attention X = tanh(enc_projT + decT) stays
[a, b*n] with per-b tensor_scalar adds (DVE/Pool) feeding ACT tanh, the
M=8-redundant diag-matmul scores trick, and per-b n=1 context matmuls.
enc_b + dec_b are folded into enc_projT at setup; energy_b is dropped
(softmax shift-invariant); fc bias is added on host.
"""

import numpy as np

import concourse.bass as bass
import concourse.bacc as bacc
import concourse.tile as tile
from concourse import mybir
from concourse.bass_utils import run_bass_kernel_spmd

F16 = mybir.dt.float16
F32 = mybir.dt.float32

B, N, H, E, A, V, L = 64, 196, 512, 512, 512, 20000, 20
T = L - 1            # 19 decode steps
NC = 8               # cores
BS = B // NC         # 8 batch rows per core
BN = BS * N          # 1568
BT = T * BS          # 152 rows, t-major (row = t*8 + b)
VC = 500             # fc vocab chunk width
NCH = V // VC        # 40
NPRE = 19            # prefetched fc chunks

# gate reorder [i,f,g,o] -> [i,f,o,g] so tanh(0.5*x) covers cols 0:1536
PERM = np.concatenate([np.arange(0, H), np.arange(H, 2 * H),
                       np.arange(3 * H, 4 * H), np.arange(2 * H, 3 * H)])

TANH = mybir.ActivationFunctionType.Tanh
EXP = mybir.ActivationFunctionType.Exp
ADD = mybir.AluOpType.add
MULT = mybir.AluOpType.mult


def prep_core(core, inputs):
    """Per-core numpy input dict (shard + transpose + cast only)."""
    f32 = np.float32
    bsl = slice(core * BS, (core + 1) * BS)
    enc = np.asarray(inputs["encoder_outputs"][bsl], f32)      # [8,196,512]

    enc_t = np.ascontiguousarray(enc.reshape(BN, H).T).astype(np.float16)
    enc_r = np.zeros((2 * BS, 128, H), np.float16)
    for b in range(BS):
        enc_r[2 * b, :128] = enc[b, :128]
        enc_r[2 * b + 1, :N - 128] = enc[b, 128:]

    caps = np.asarray(inputs["captions"][bsl])[:, :T]          # [8,19]
    es = np.asarray(inputs["emb"], f32)[caps]                  # [8,19,512]
    emb_flat = es.transpose(1, 0, 2).reshape(BT, E)            # t-major rows
    emb_cat = np.concatenate(
        [emb_flat.T, np.ones((1, BT), f32)], 0).astype(np.float16)

    wih = np.asarray(inputs["W_ih"], f32)[PERM]                # [2048,1024]
    whh = np.asarray(inputs["W_hh"], f32)[PERM]
    bias = (np.asarray(inputs["b_ih"], f32) +
            np.asarray(inputs["b_hh"], f32))[PERM]
    wihxb_t = np.concatenate(
        [wih[:, :E].T, bias[None, :]], 0)                      # [513,2048]
    wc_t = np.concatenate([wih[:, E:].T, 0.5 * whh.T], 0)      # [1024,2048]
    # pre-double the g-gate columns so the pointwise uses one tanh(0.5 x)
    wihxb_t[:, 3 * H:] *= 2.0
    wc_t[:, 3 * H:] *= 2.0
    wihxb_t = wihxb_t.astype(np.float16)
    wc_t = wc_t.astype(np.float16)

    dec_wt = (0.5 * np.asarray(inputs["dec_W"], f32).T).astype(np.float16)
    enc_wt = np.ascontiguousarray(
        np.asarray(inputs["enc_W"], f32).T).astype(np.float16)  # [H,A]
    ebdb = np.ascontiguousarray(
        (np.asarray(inputs["enc_b"], f32) +
         np.asarray(inputs["dec_b"], f32)).reshape(4, 128).T)   # [128,4]
    ew = np.ascontiguousarray(
        np.asarray(inputs["energy_W"], f32)[0].reshape(4, 128).T
    ).astype(np.float16)                                        # [128,4]
    fcw_t = np.ascontiguousarray(
        0.5 * np.asarray(inputs["fc_W"], f32).T).astype(np.float16)
    id8 = np.eye(8, dtype=np.float16)

    return {"enc_t": enc_t, "enc_r": enc_r, "emb_cat": emb_cat,
            "wihxb_t": wihxb_t, "wc_t": wc_t, "dec_wt": dec_wt,
            "enc_wt": enc_wt, "ebdb": ebdb, "ew": ew,
            "fcw_t": fcw_t, "id8": id8}


def _bcast(ap, n):
    """Append an innermost step-0 (broadcast) dim of size n to an AP."""
    return bass.AP(tensor=ap.tensor, offset=ap.offset,
                   ap=list(ap.ap) + [[0, n]])


def build_program():
    nc = bacc.Bacc("TRN2", target_bir_lowering=False, debug=False,
                   num_devices=NC)
    d_enc_t = nc.dram_tensor("enc_t", [H, BN], F16, kind="ExternalInput")
    d_enc_r = nc.dram_tensor("enc_r", [2 * BS, 128, H], F16,
                             kind="ExternalInput")
    d_emb = nc.dram_tensor("emb_cat", [E + 1, BT], F16, kind="ExternalInput")
    d_wx = nc.dram_tensor("wihxb_t", [E + 1, 4 * H], F16,
                          kind="ExternalInput")
    d_wc = nc.dram_tensor("wc_t", [2 * H, 4 * H], F16, kind="ExternalInput")
    d_dwt = nc.dram_tensor("dec_wt", [H, A], F16, kind="ExternalInput")
    d_ewt = nc.dram_tensor("enc_wt", [H, A], F16, kind="ExternalInput")
    d_ebdb = nc.dram_tensor("ebdb", [128, 4], F32, kind="ExternalInput")
    d_ew = nc.dram_tensor("ew", [128, 4], F16, kind="ExternalInput")
    d_fcw = nc.dram_tensor("fcw_t", [H, V], F16, kind="ExternalInput")
    d_id8 = nc.dram_tensor("id8", [8, 8], F16, kind="ExternalInput")
    d_out = nc.dram_tensor("logits", [BT, V], F16, kind="ExternalOutput")

    with tile.TileContext(nc) as tc:
        _build_body(nc, tc, d_enc_t, d_enc_r, d_emb, d_wx, d_wc, d_dwt,
                    d_ewt, d_ebdb, d_ew, d_fcw, d_id8, d_out)
    nc.compile()
    return nc


def _build_body(nc, tc, d_enc_t, d_enc_r, d_emb, d_wx, d_wc, d_dwt, d_ewt,
                d_ebdb, d_ew, d_fcw, d_id8, d_out):
    with tc.tile_pool(name="res", bufs=1) as res:
        # -------- residents --------
        ept = res.tile([128, 4, BN], F16)        # enc_projT (+enc_b+dec_b)
        enr = res.tile([128, 2 * BS, H], F16)    # enc rows [n-tiles, h]
        wc = res.tile([128, 8, 4 * H], F16)      # [ctx;h] gate weights^T
        dwt = res.tile([128, 4, A], F16)         # 0.5 dec_W^T
        wx = res.tile([128, 5, 4 * H], F16)      # W_ih_x^T (+bias row)
        emb = res.tile([128, 5, BT], F16)        # emb_cat k-tiles (+ones row)
        hallt = res.tile([128, 4, BT], F16)      # h2^T, all steps
        ewm = res.tile([128, 4, 64], F16)        # diag: [:,at,bl*8+bl]=ew
        id8 = res.tile([8, 8], F16)
        c2 = res.tile([128, 4, 8], F32)          # 2c, transposed
        ebdb = res.tile([128, 4], F32)
        fcpre = res.tile([128, NPRE, 4, VC], F16)

        # tiny first, then DMAs ordered by first use
        nc.sync.dma_start(out=id8[:, :], in_=d_id8[:, :])
        nc.sync.dma_start(out=ebdb[:, :], in_=d_ebdb[:, :])
        ew_col = res.tile([128, 4], F16)
        nc.sync.dma_start(out=ew_col[:, :], in_=d_ew[:, :])
        nc.vector.memset(ewm[:, :, :], 0.0)
        for at in range(4):
            col = ew_col[:, at:at + 1]
            dg = ewm[:, at, :]
            nc.vector.tensor_copy(
                out=bass.AP(tensor=dg.tensor, offset=dg.offset,
                            ap=[dg.ap[0], [9, 8]]),
                in_=bass.AP(tensor=col.tensor, offset=col.offset,
                            ap=[col.ap[0], [0, 8]]))
        nc.vector.memset(c2[:, :, :], 0.0)
        z1 = res.tile([1, 128], F16)
        nc.vector.memset(z1[:, :], 0.0)
        zf = res.tile([1, 128], F32)
        nc.vector.memset(zf[:, :], 0.0)


        def flat(tile_ap, ncols):
            return tile_ap.rearrange("p a b -> p (a b)")
        def bank_open(ap, ncols, dep=None):
            # full-tile zero matmul: zeroes the 2KB bank and starts its single
            # accumulation group. lhsT is all-zero so the rhs (an optional
            # SBUF tile produced by the op this must execute after) only
            # carries a read dependency.
            rhs = z1[0:1, 0:ncols] if dep is None else dep
            lhs = zf if rhs.dtype == F32 else z1
            nc.tensor.matmul(ap, lhs[0:1, 0:128], rhs,
                             start=True, stop=False)

        def bank_close(ap, ncols, dep=None):
            rhs = z1[0:1, 0:ncols] if dep is None else dep
            lhs = zf if rhs.dtype == F32 else z1
            nc.tensor.matmul(ap, lhs[0:1, 0:128], rhs,
                             start=False, stop=True)

        # -------- setup: enc_projT (+enc_b+dec_b) --------
        with tc.tile_pool(name="se", bufs=1) as se, \
             tc.tile_pool(name="sep", bufs=2, space="PSUM") as sep:
            et = se.tile([128, 4, BN], F16)
            ewt = se.tile([128, 4, A], F16)
            nc.sync.dma_start(
                out=ewt[:, :, :],
                in_=d_ewt[:, :].rearrange("(k p) a -> p k a", p=128))
            for ch in range(4):                      # chunked: MMs start early
                nc.sync.dma_start(
                    out=et[:, :, ch * 392:(ch + 1) * 392],
                    in_=d_enc_t[:, ch * 392:(ch + 1) * 392].rearrange(
                        "(k p) n -> p k n", p=128))
            nc.sync.dma_start(out=enr[:, :, :],
                              in_=d_enc_r[:, :, :].rearrange("j p h -> p j h"))
            nc.sync.dma_start(
                out=emb[:, 0:4, :],
                in_=d_emb[0:512, :].rearrange("(k p) t -> p k t", p=128))
            nc.sync.dma_start(out=emb[0:1, 4, :], in_=d_emb[512:513, :])
            nc.sync.dma_start(
                out=wx[:, 0:4, :],
                in_=d_wx[0:512, :].rearrange("(k p) g -> p k g", p=128))
            nc.sync.dma_start(out=wx[0:1, 4, :], in_=d_wx[512:513, :])
            # wc: ctx half (k 0:4) first (needed at t0), then h half
            nc.sync.dma_start(
                out=wc[:, 0:4, :],
                in_=d_wc[0:512, :].rearrange("(k p) g -> p k g", p=128))
            nc.sync.dma_start(
                out=wc[:, 4:8, :],
                in_=d_wc[512:1024, :].rearrange("(k p) g -> p k g", p=128))
            nc.sync.dma_start(
                out=dwt[:, :, :],
                in_=d_dwt[:, :].rearrange("(k p) a -> p k a", p=128))
            for ch in range(NPRE):                # stream during recurrence
                nc.sync.dma_start(
                    out=fcpre[:, ch, :, :],
                    in_=d_fcw[:, ch * VC:(ch + 1) * VC].rearrange(
                        "(k p) v -> p k v", p=128))
            for at in range(4):                      # a-tile = out m-tile
                for ch in range(4):                  # 1568 = 4*392
                    pt = sep.tile([128, 392], F32, tag="sep")
                    for k in range(4):
                        nc.tensor.matmul(
                            pt[:, :],
                            ewt[:, k, at * 128:(at + 1) * 128],
                            et[:, k, ch * 392:(ch + 1) * 392],
                            start=(k == 0), stop=(k == 3))
                    nc.vector.tensor_scalar_add(
                        out=ept[:, at, ch * 392:(ch + 1) * 392],
                        in0=pt[:, :], scalar1=ebdb[:, at:at + 1])

        # fc m0 weave: MM emission points fill PE idle (pre-scores window and
        # the pointwise/head window); drains (psum->sbuf copy + DMA out) go on
        # DVE after the adds and on ACT after thc, when those engines idle
        NW = 17              # chunks woven into the recurrence

        # -------- recurrence (+ fc m-tile 0 woven into steps 16-18) --------
        with tc.tile_pool(name="pdx", bufs=1, space="PSUM") as pdx, \
             tc.tile_pool(name="pgts", bufs=1, space="PSUM") as pgts, \
             tc.tile_pool(name="psc", bufs=1, space="PSUM") as pscp, \
             tc.tile_pool(name="ptp", bufs=1, space="PSUM") as ptp, \
             tc.tile_pool(name="pf", bufs=4, space="PSUM") as pfp, \
             tc.tile_pool(name="stp", bufs=2) as stp, \
             tc.tile_pool(name="fo", bufs=4) as fop, \
             tc.tile_pool(name="xp", bufs=1) as xp:

            fc_pending = []

            def fc_mm(ch, m, fws):
                mr = 128 if m == 0 else BT - 128
                pf = pfp.tile([128, VC], F32, tag="pf")
                for k in range(4):
                    nc.tensor.matmul(
                        pf[0:mr, :], hallt[:, k, m * 128:m * 128 + mr],
                        fws[:, k, :], start=(k == 0), stop=(k == 3))
                fc_pending.append((pf, ch, m, mr))

            def fc_drain(engs, limit=None, dma=None):
                ndrain = len(fc_pending) if limit is None else \
                    min(limit, len(fc_pending))
                for i in range(ndrain):
                    pf, ch, m, mr = fc_pending[i]
                    eng = engs[i % len(engs)]
                    fo = fop.tile([128, VC], F16, tag="fo")
                    if eng is nc.scalar:
                        eng.copy(out=fo[0:mr, :], in_=pf[0:mr, :])
                    else:
                        eng.tensor_copy(out=fo[0:mr, :], in_=pf[0:mr, :])
                    (dma or nc.sync).dma_start(
                        out=d_out[m * 128:m * 128 + mr,
                                  ch * VC:(ch + 1) * VC],
                        in_=fo[0:mr, :])
                del fc_pending[:ndrain]

            def fc_chunk(ch, m, fws, eng=None):
                fc_mm(ch, m, fws)
                fc_drain([eng if eng is not None else nc.vector])

            fc_feed = iter(range(NW))

            def fc_fill():
                while len(fc_pending) < 4:
                    ch = next(fc_feed, None)
                    if ch is None:
                        return
                    fc_mm(ch, 0, fcpre[:, ch, :, :])

            for t in range(T):
                def hpk(k, _t=t):
                    return hallt[:, k, (_t - 1) * 8:_t * 8]

                # ---- decT = (.5 dec_W)^T @ 2h, a-tiles [128, 8] ----
                if t > 0:
                    dps = pdx.tile([128, 4, 8], F32, tag="dx",
                                   name=f"dec{t}")
                    dect = stp.tile([128, 4, 8], F32, tag="dect")
                    # single group per 2KB bank, opened/closed by dummy
                    # full-tile zero matmuls (WAW-ordered by the scheduler)
                    bank_open(flat(dps, 32), 32,
                              dep=hallt[0:1, :, (t - 1) * 8:t * 8])
                    for at in range(4):
                        for k in range(4):
                            nc.tensor.matmul(
                                dps[:, at, :],
                                dwt[:, k, at * 128:(at + 1) * 128],
                                hpk(k), start=False, stop=False)
                    bank_close(flat(dps, 32), 32)
                    # at3 first: Pool's long bcast add depends on it
                    nc.vector.tensor_copy(out=dect[:, 3, :],
                                          in_=dps[:, 3, :])

                    # ---- gates: h-part + x-part (only need h(t-1)) ----
                    gps = pgts.tile([128, 16, 8], F32, tag="gates",
                                    name=f"g{t}")
                    bank_open(flat(gps, 128), 128)
                    for g in range(16):
                        gsl = slice(g * 128, (g + 1) * 128)
                        for k in range(4):
                            nc.tensor.matmul(gps[:, g, :], wc[:, 4 + k, gsl],
                                             hpk(k), start=False, stop=False)
                        for k in range(5):
                            kr = 128 if k < 4 else 1
                            nc.tensor.matmul(
                                gps[:, g, :], wx[0:kr, k, gsl],
                                emb[0:kr, k, t * 8:(t + 1) * 8],
                                start=False, stop=False)
                else:
                    gps = pgts.tile([128, 16, 8], F32, tag="gates", name="g0")
                    bank_open(flat(gps, 128), 128)
                    for g in range(16):
                        gsl = slice(g * 128, (g + 1) * 128)
                        for k in range(5):
                            kr = 128 if k < 4 else 1
                            nc.tensor.matmul(
                                gps[:, g, :], wx[0:kr, k, gsl],
                                emb[0:kr, k, 0:8],
                                start=False, stop=False)

                # fc weave: MMs run in the PE idle before the scores unblock
                if t >= 16:
                    fc_fill()

                # ---- X = tanh(ept + decT bcast); scores ----
                psc = pscp.tile([8, N], F32, tag="sc")
                for at in range(4):
                    xt = xp.tile([128, BN], F16, tag=f"x{at}",
                                 name=f"xt{t}_{at}")
                    if t == 0:
                        nc.scalar.activation(out=xt[:, :],
                                             in_=ept[:, at, :], func=TANH)
                    else:
                        if at == 3:
                            # GPSIMD cannot read PSUM: dect[:, 3] was copied
                            # out first thing so this 3.2us add starts early
                            nc.gpsimd.tensor_add(
                                out=xt[:, :].rearrange(
                                    "p (b n) -> p b n", n=N),
                                in0=ept[:, at, :].rearrange(
                                    "p (b n) -> p b n", n=N),
                                in1=_bcast(dect[:, 3, :], N))
                        else:
                            # per-at psum->sbuf copy right before its adds,
                            # so adds for a-tile 0 start asap
                            nc.vector.tensor_copy(out=dect[:, at, :],
                                                  in_=dps[:, at, :])
                            for b in range(BS):
                                nc.vector.tensor_scalar_add(
                                    out=xt[:, b * N:(b + 1) * N],
                                    in0=ept[:, at, b * N:(b + 1) * N],
                                    scalar1=dect[:, at, b:b + 1])
                        nc.scalar.activation(out=xt[:, :], in_=xt[:, :],
                                             func=TANH)
                    for bl in range(BS):
                        nc.tensor.matmul(
                            psc[:, :], ewm[:, at, bl * 8:(bl + 1) * 8],
                            xt[:, bl * N:(bl + 1) * N],
                            start=(at == 0 and bl == 0),
                            stop=(at == 3 and bl == 7))

                # drain fc chunks whose MMs were emitted earlier (DVE is past
                # the adds by now)
                if t >= 16:
                    fc_drain([nc.vector], limit=3)

                # ---- softmax + attw^T ----
                atw = stp.tile([8, N], F16, tag="atw")
                zs = stp.tile([8, 1], F32, tag="zs")
                nc.scalar.activation(out=atw[:, :], in_=psc[:, :],
                                     func=EXP, accum_out=zs[:, 0:1])
                rz = stp.tile([8, 1], F32, tag="rz")
                nc.vector.reciprocal(out=rz[:, :], in_=zs[:, :])
                atwn = stp.tile([8, N], F16, tag="atwn")
                nc.vector.tensor_scalar_mul(out=atwn[:, :], in0=atw[:, :],
                                            scalar1=rz[:, :])
                p12 = ptp.tile([128, 2, 8], F16, tag="tp")
                nc.tensor.transpose(p12[:, 0, :], atwn[:, 0:128], id8[:, :])
                nc.tensor.transpose(p12[0:N - 128, 1, :], atwn[:, 128:N],
                                    id8[:, :])
                awt = stp.tile([128, 2, 8], F16, tag="awt")
                nc.vector.tensor_copy(out=awt[:, 0, :], in_=p12[:, 0, :])
                nc.vector.tensor_copy(out=awt[0:N - 128, 1, :],
                                      in_=p12[0:N - 128, 1, :])

                # ---- ctxT a-tiles [128, 8] via per-b n=1 matmuls ----
                cps = pdx.tile([128, 4, 8], F32, tag="dx", name=f"ctx{t}")
                bank_open(flat(cps, 32), 32, dep=atwn[0:1, 0:32])
                for b in range(BS):
                    for at in range(4):
                        asl = slice(at * 128, (at + 1) * 128)
                        nc.tensor.matmul(
                            cps[:, at, b:b + 1], enr[0:128, 2 * b, asl],
                            awt[0:128, 0, b:b + 1],
                            start=False, stop=False)
                        nc.tensor.matmul(
                            cps[:, at, b:b + 1],
                            enr[0:N - 128, 2 * b + 1, asl],
                            awt[0:N - 128, 1, b:b + 1],
                            start=False, stop=False)
                bank_close(flat(cps, 32), 32)
                ctxt = stp.tile([128, 4, 8], F16, tag="ctxt")
                nc.vector.tensor_copy(out=ctxt[:, :, :], in_=cps[:, :, :])

                # ---- gates: ctx-part ----
                for g in range(16):
                    gsl = slice(g * 128, (g + 1) * 128)
                    for k in range(4):
                        nc.tensor.matmul(gps[:, g, :], wc[:, k, gsl],
                                         ctxt[:, k, :],
                                         start=False, stop=False)
                bank_close(flat(gps, 128), 128)

                # fc m0 weave MMs: PE is idle during pointwise + next head
                if t >= 16:
                    fc_fill()

                # ---- pointwise, transposed; g-gates pre-doubled on host ----
                th = stp.tile([128, 16, 8], F16, tag="th")
                nc.scalar.activation(out=th[:, :, :], in_=gps[:, :, :],
                                     func=TANH, scale=0.5)
                a2 = stp.tile([128, 4, 8], F32, tag="a2")
                nc.vector.scalar_tensor_tensor(
                    out=a2[:, :, :], in0=th[:, 4:8, :], scalar=1.0,
                    in1=c2[:, :, :], op0=ADD, op1=MULT)
                bb = stp.tile([128, 4, 8], F32, tag="bb")
                nc.vector.scalar_tensor_tensor(
                    out=bb[:, :, :], in0=th[:, 0:4, :], scalar=1.0,
                    in1=th[:, 12:16, :], op0=ADD, op1=MULT)
                nc.vector.scalar_tensor_tensor(
                    out=c2[:, :, :], in0=a2[:, :, :], scalar=0.5,
                    in1=bb[:, :, :], op0=MULT, op1=ADD)
                thc = stp.tile([128, 4, 8], F16, tag="thc")
                nc.scalar.activation(out=thc[:, :, :], in_=c2[:, :, :],
                                     func=TANH, scale=0.5)
                nc.vector.scalar_tensor_tensor(
                    out=hallt[:, :, t * 8:(t + 1) * 8], in0=th[:, 8:12, :],
                    scalar=1.0, in1=thc[:, :, :], op0=ADD, op1=MULT)
                # second drain point: ACT idles between thc and next tanh
                if t >= 16:
                    fc_drain([nc.scalar], limit=2)

            # -------- fc tail: interleave streamed chunks (m0+m1, heavy PE)
            # with prefetched m1-only chunks so the stream DMA stays hidden --
            with tc.tile_pool(name="fw", bufs=3) as fwp:
                pre_m1 = list(range(NPRE))           # m1 of prefetched
                pre_m0 = list(range(NW, NPRE))       # m0 not yet woven
                stream = list(range(NPRE, NCH))
                jobs = []
                while pre_m1 or pre_m0 or stream:
                    if stream:
                        jobs.append(("s", stream.pop(0)))
                    if pre_m0:
                        jobs.append(("p0", pre_m0.pop(0)))
                    if pre_m1:
                        jobs.append(("p1", pre_m1.pop(0)))
                def tail_chunk(ch, m, fws, eng):
                    fc_mm(ch, m, fws)
                    fc_drain([eng], dma=nc.gpsimd)

                for i, (kind, ch) in enumerate(jobs):
                    eng = nc.scalar if i % 3 == 2 else nc.vector
                    if kind == "s":
                        fws = fwp.tile([128, 4, VC], F16, tag="fw",
                                       name=f"fw{ch}")
                        nc.sync.dma_start(
                            out=fws[:, :, :],
                            in_=d_fcw[:, ch * VC:(ch + 1) * VC].rearrange(
                                "(k p) v -> p k v", p=128))
                        tail_chunk(ch, 0, fws, nc.scalar)
                        tail_chunk(ch, 1, fws, nc.vector)
                    elif kind == "p0":
                        tail_chunk(ch, 0, fcpre[:, ch, :, :], eng)
                    else:
                        tail_chunk(ch, 1, fcpre[:, ch, :, :], eng)


_PROGRAM = None


def kernel(**inputs) -> np.ndarray:
    global _PROGRAM
    if _PROGRAM is None:
        _PROGRAM = build_program()
    in_maps = [prep_core(c, inputs) for c in range(NC)]
    res = run_bass_kernel_spmd(_PROGRAM, in_maps, core_ids=list(range(NC)))
    fcb = np.asarray(inputs["fc_b"], np.float32)
    out = np.zeros((B, L, V), np.float32)
    for c in range(NC):
        lg = res.results[c]["logits"].astype(np.float32).reshape(T, BS, V)
        out[c * BS:(c + 1) * BS, 1:, :] = lg.transpose(1, 0, 2) + fcb
    return out


if __name__ == "__main__":
    import reference
    ins = {k: np.asarray(v) for k, v in reference.setup_inputs().items()}
    got = kernel(**ins)
    exp = np.asarray(reference.reference(**reference.setup_inputs()))
    err = np.abs(got - exp).max() / (np.abs(exp).max() + 1e-12)
    print("Relative error:", err)
